# revision 67
# baseline (speedup 1.0000x reference)
"""Trainium2 Bass kernel for nn_ChannelWiseQuantumKernel (v3).

Method (per-pixel phase precompute + compact z/w state recurrence):

The per-position data RY gates are diagonal phase gates in the (SH)-transformed
basis: the 16 phase angles per patch-position are SHIFTED READS of a per-PIXEL
field alpha_j(img, h, w) = sum_ch +-theta_ch/2 (a 4->16 signed channel mix).
All trig is computed once per pixel: PhiC = -cos(alpha)/4, PhiS = -sin(alpha)/4
stored as fp16 pixel arrays [128, 2, 64, 64] (partitions = kernel(2) x img(2)
x state(32)).

State: compact 32 real values per (kernel, img): y = [yR(16); yI(16)], all
four (k, img) groups packed in 128 partitions. Per position p:
    z = PhiC_view(p) * y ;  w = PhiS_view(p) * y      (one DVE TT, fp16 2x
                                                       mode, broadcast y)
    y' = M1_p @ z + M2_p @ w                          (two accumulated fp16
                                                       128x128 matmuls)

v3 structural changes over the 88us v2 baseline (measured ~80-85us):
 * ALL PSUM lives in one [128, 8, 512] fp32 mega-tile (bank c = chunk c's
   state accumulator, in-place across positions via subtile hazards).
 * 62 output rows split into 8 chunks of 8 rows (chunk 7: 6); chunk PAIRS
   form 4 independent chains, each sharing one ACT downconvert (a single
   [128, 2banks, 496] -> fp16 copy) and one 16-row 2x-mode DVE multiply,
   halving per-instruction init overhead vs per-chunk ops. Chains run in
   lockstep rounds (q order 0,2,3,1) - engine-throughput-bound, with chain
   latency (~3.5us) hidden across rounds.
 * The PE p-state is held at full clock (2.4 GHz, 207ns/496-col matmul
   instead of 413ns at the mid p-state) by a continuous stream of filler
   matmuls: any PE idle gap resets the 3us ramp, so the fillers are
   load-bearing. 496-col fillers go to bank 3 (pair 1 starts last) during
   the pixel phase; 128-col accumulate-only fillers go to the spare cols
   [384:512] of bank 7 (chunk 7 only uses 372) during the sweeps.
 * Pixel phase runs in 4 superchunks (1024 px): 2 alpha matmuls into
   alternating bank pairs (4,5)/(6,7) for 2-deep pipelining, 2 merged
   [128,1024] SIN activations, 1 merged CUBE; a warm-up SIN at t=0 pulls
   the activation-table load out of the critical path. Pair 0's first
   sweep unit is emitted mid-pixel-phase to overlap the pixel tail.
 * GpSimd runs NO compute: it shares SBUF read/write ports with the DVE,
   so its TTs would stall vector work (only DMA kicks go there).
 * Measurement (square -> +-1 matmul -> fp16 copy -> DMA) is emitted inside
   the last round per pair and alternates ACT/DVE for the square/copy.
"""

import sys

sys.path.insert(0, "/opt/trn_rl_repo")

import numpy as np

import concourse.bacc as bacc
import concourse.bass as bass
import concourse.tile as tile
from concourse import mybir
from concourse.bass_utils import run_bass_kernel_spmd

# ---------------- problem constants ----------------
IN_CH = 4
KSZ = 3
NPOS = 9
DIM = 16
B = 16
HW = 64
OH = HW - KSZ + 1  # 62
P = OH * OH  # 3844 patches per image
N_CORES = 8
NPIX = HW * HW  # 4096
NCH = 8  # 8 chunks of 8 output rows (chunk 7 has 6 valid rows)
NPAIR = 4

# knobs
PIX_DUM = 7  # 496-col PE fillers per pixel-superchunk slot
SWEEP_DUM = 5  # 128-col PE fillers after each pair's matmuls
# pair units: (first chunk, end chunk, rows); 4 independent chains
PAIRS = [(0, 2, 16), (2, 4, 16), (4, 6, 16), (6, 8, 14)]

# wm (fp16 weight matrix) column layout
A_ALPHA = 0
A_P0C = 128
A_P0S = 256
A_M = 384  # 16 x 128 cols: (p-1)*256 + {0 (M1), 128 (M2)}
A_Z = A_M + 16 * 128  # 2432
WCOLS = A_Z + 16  # 2448

# ---------------- host-side constant math (weights-only, O(1)) ----------------
_H = np.array([[1, 1], [1, -1]], dtype=np.complex128) / np.sqrt(2)
_S = np.array([[1, 0], [0, 1j]], dtype=np.complex128)
_A1 = _S @ _H


def _kron_n(mats):
    out = np.array([[1.0 + 0j]])
    for m in mats:
        out = np.kron(out, m)
    return out


_AA = _kron_n([_A1] * IN_CH)
_U1 = _A1.conj().T @ (np.array([1.0, 1.0]) / np.sqrt(2))
_YINIT = _kron_n([_U1.reshape(2, 1)] * IN_CH).reshape(DIM)

_SGN = np.array(
    [[2 * ((i >> (3 - ch)) & 1) - 1 for i in range(DIM)] for ch in range(IN_CH)],
    dtype=np.float64,
)
_SIGMA = np.array(
    [[1 - 2 * ((i >> (3 - q)) & 1) for i in range(DIM)] for q in range(IN_CH)],
    dtype=np.float64,
)


def _rx(t):
    c, s = np.cos(t / 2), np.sin(t / 2)
    return np.array([[c, -1j * s], [-1j * s, c]])


def _ry(t):
    c, s = np.cos(t / 2), np.sin(t / 2)
    return np.array([[c, -s], [s, c]])


def _rz(t):
    e = np.exp(-0.5j * t)
    return np.array([[e, 0], [0, np.conj(e)]])


def _embed(U, q):
    mats = [np.eye(2, dtype=complex)] * IN_CH
    mats[q] = U
    return _kron_n(mats)


def _cx(cq, tq):
    M = np.zeros((DIM, DIM), dtype=complex)
    for i in range(DIM):
        bits = [(i >> (3 - q)) & 1 for q in range(4)]
        j = i
        if bits[cq] == 1:
            bits2 = bits.copy()
            bits2[tq] ^= 1
            j = sum(b << (3 - q) for q, b in enumerate(bits2))
        M[j, i] = 1
    return M


def _build_G(w_flat):
    w = np.float64(w_flat).reshape(NPOS, 1, IN_CH, 3)
    Gs = []
    for pos in range(NPOS):
        U = np.eye(DIM, dtype=complex)
        for q in range(IN_CH):
            R = _rz(w[pos, 0, q, 2]) @ _ry(w[pos, 0, q, 1]) @ _rx(w[pos, 0, q, 0])
            U = _embed(R, q) @ U
        for q in range(IN_CH - 1):
            U = _cx(q, q + 1) @ U
        U = _cx(IN_CH - 1, 0) @ U
        Gs.append(_AA.conj().T @ U @ _AA)
    G8f = _AA @ Gs[8]
    return Gs, G8f


def _build_wm(weights):
    """All matmul lhsT constants, fp16 [128, WCOLS].

    Partition groups g = k*2 + img occupy partitions [32g, 32g+32)."""
    wm = np.zeros((128, WCOLS), dtype=np.float64)
    # alpha lhsT: rows (img*4+ch) 0..8, cols (g, j32); emits alpha/3 directly
    for k in range(2):
        for img in range(2):
            g = k * 2 + img
            for ch in range(IN_CH):
                for j in range(32):
                    wm[img * 4 + ch, A_ALPHA + g * 32 + j] = _SGN[ch, j % 16] / 6.0
    for k in range(2):
        Gs, G8f = _build_G(weights[k])
        G0c = Gs[0] @ np.diag(_YINIT)
        Pm, Qm = G0c.real, G0c.imag
        Ac = np.vstack([Pm, Qm])  # 32x16: c16 -> [y1R; y1I]
        As = np.vstack([-Qm, Pm])
        for img in range(2):
            g = k * 2 + img
            r0 = g * 32
            # pos0: lhsT[r0+j, A_P0C + r0+o] = -4*Ac[o, j] (j<16)
            wm[r0 : r0 + 16, A_P0C + r0 : A_P0C + r0 + 32] = -4.0 * Ac.T
            wm[r0 : r0 + 16, A_P0S + r0 : A_P0S + r0 + 32] = -4.0 * As.T
        for p in range(1, 9):
            G = G8f if p == 8 else Gs[p]
            Gr, Gi = G.real, G.imag
            M1 = np.block([[Gr, -Gi], [Gi, Gr]])
            M2 = np.block([[-Gi, -Gr], [Gr, -Gi]])
            col = A_M + (p - 1) * 256
            for img in range(2):
                g = k * 2 + img
                r0 = g * 32
                wm[r0 : r0 + 32, col + r0 : col + r0 + 32] = -4.0 * M1.T
                wm[r0 : r0 + 32, col + 128 + r0 : col + 128 + r0 + 32] = -4.0 * M2.T
        # Z lhsT: out row r = k*8 + img*4 + q
        for img in range(2):
            g = k * 2 + img
            r0 = g * 32
            for q in range(IN_CH):
                for j in range(32):
                    wm[r0 + j, A_Z + k * 8 + img * 4 + q] = _SIGMA[q, j % 16]
    return wm.astype(np.float16)


# ---------------- custom fused DVE op: out = ((v^2 - 3/4) * v) * y ----------
_CUBE_OP = None


def _register_cube_mul():
    """Register the fused triple-angle multiply as a custom DVE op.

    One VectorE instruction computes ((v*v - c0) * v) * y, which applies the
    full range-reduced phase factor to the state in a single pass."""
    global _CUBE_OP
    if _CUBE_OP is not None:
        return _CUBE_OP
    import concourse.dve_ops as dve_ops

    for o in dve_ops.OPS:
        if o.name == "CUBE_MUL_ANT":
            _CUBE_OP = o
            return o
    from concourse.dve_ops import DveOp
    from concourse.dve_spec import C0, Spec, Src0, Src1, lower
    from concourse.dve_uop import DveOpSpec

    body = ((Src0 * Src0 - C0) * Src0) * Src1
    spec = Spec(
        body=body,
        reference=lambda in0, in1, c0, c1, c2: (
            ((in0.astype(np.float32) * in0 - c0) * in0) * in1
        ),
    )
    row = max(dve_ops._SUB_OPCODE_FOR_NAME.values()) + 1
    shas = {}
    for ver in ("v3", "v4"):
        uops = lower(spec, ver=ver)
        shas[ver] = DveOpSpec(
            name="CUBE_MUL_ANT", opcode=row, uops=uops, rd1_en=True
        ).sha(ver)
    op = DveOp("CUBE_MUL_ANT", spec, subdim=False, uops_sha=shas)
    dve_ops.OPS.append(op)
    dve_ops._SUB_OPCODE_FOR_NAME[op.name] = row
    dve_ops.CUSTOM_DVE_SPECS[op.name] = spec
    _CUBE_OP = op
    return op


_SQ_OP = None


def _register_square():
    """Single-source DVE square (PSUM-in, fp16-out) for the measurement."""
    global _SQ_OP
    if _SQ_OP is not None:
        return _SQ_OP
    import concourse.dve_ops as dve_ops

    for o in dve_ops.OPS:
        if o.name == "SQ_ANT":
            _SQ_OP = o
            return o
    from concourse.dve_ops import DveOp
    from concourse.dve_spec import Spec, Src0, lower
    from concourse.dve_uop import DveOpSpec

    spec = Spec(
        body=Src0 * Src0,
        reference=lambda in0, in1, c0, c1, c2: in0.astype(np.float32) ** 2,
    )
    row = max(dve_ops._SUB_OPCODE_FOR_NAME.values()) + 1
    shas = {}
    for ver in ("v3", "v4"):
        uops = lower(spec, ver=ver)
        shas[ver] = DveOpSpec(
            name="SQ_ANT", opcode=row, uops=uops, rd1_en=False
        ).sha(ver)
    op = DveOp("SQ_ANT", spec, subdim=False, uops_sha=shas)
    dve_ops.OPS.append(op)
    dve_ops._SUB_OPCODE_FOR_NAME[op.name] = row
    dve_ops.CUSTOM_DVE_SPECS[op.name] = spec
    _SQ_OP = op
    return op


# ---------------- device program ----------------
_PROGRAM_CACHE = {}

f16 = mybir.dt.float16
f32 = mybir.dt.float32


def _chunk_nr(c):
    return 6 if c == NCH - 1 else 8


def _chunk_c0(c):
    return 496 * c


def _build_program():
    key = "v3"
    if key in _PROGRAM_CACHE:
        return _PROGRAM_CACHE[key]

    nc = bacc.Bacc("TRN2", target_bir_lowering=False, debug=False)
    x_d = nc.dram_tensor("xin", [8, NPIX], f16, kind="ExternalInput").ap()
    wm_d = nc.dram_tensor("wm", [128, WCOLS], f16, kind="ExternalInput").ap()
    bias_d = nc.dram_tensor("bias32", [128, 2], f32, kind="ExternalInput").ap()
    z_d = nc.dram_tensor("zout", [16, P], f16, kind="ExternalOutput").ap()

    CUBE = _register_cube_mul()
    SQOP = _register_square()
    SIN = mybir.ActivationFunctionType.Sin
    COPY = mybir.ActivationFunctionType.Copy
    SQUARE = mybir.ActivationFunctionType.Square

    with tile.TileContext(nc) as tc:
        from contextlib import ExitStack

        with ExitStack() as ctx:
            const_pool = ctx.enter_context(tc.tile_pool(name="const", bufs=1))
            v_pool = ctx.enter_context(tc.tile_pool(name="v", bufs=2))
            ysb_pool = ctx.enter_context(tc.tile_pool(name="ysb", bufs=4))
            m_pool = ctx.enter_context(tc.tile_pool(name="m", bufs=4))
            sq_pool = ctx.enter_context(tc.tile_pool(name="sq", bufs=2))
            ps_pool = ctx.enter_context(
                tc.tile_pool(name="ps", bufs=1, space="PSUM")
            )

            wm_sb = const_pool.tile([128, WCOLS], f16)
            x_sb = const_pool.tile([8, NPIX], f16)
            bias_sb = const_pool.tile([128, 2], f32)
            ones_sb = const_pool.tile([128, 2048], f16)
            # Phi pixel arrays: plane 0 = -cos(alpha)/4, plane 1 = -sin/4
            phi = const_pool.tile([128, 2, HW, HW], f16)
            zbuf = const_pool.tile([16, P], f16)
            # ALL of PSUM: bank c = chunk c state accumulator
            mega = ps_pool.tile([128, NCH, 512], f32)

            def wma(c0, n=128):
                return wm_sb[:, c0 : c0 + n]

            def big_dummy():
                # full-col filler into bank 3 (unowned until pair-1 p0,
                # which is deliberately scheduled last)
                nc.tensor.matmul(
                    mega[:, 3, :496],
                    ones_sb[:, 0:128],
                    ones_sb[:, 0:496],
                    start=True,
                    stop=True,
                    skip_group_check=True,
                )

            def big_dummy2():
                # filler into bank 2 during the late pixel window
                nc.tensor.matmul(
                    mega[:, 2, :496],
                    ones_sb[:, 0:128],
                    ones_sb[:, 0:496],
                    start=True,
                    stop=True,
                    skip_group_check=True,
                )

            def small_dummy():
                # accumulate-only filler into bank 7 spare cols; 32-row
                # stationary keeps the implicit weight load short
                nc.tensor.matmul(
                    mega[:, 7, 384:512],
                    ones_sb[0:32, 0:128],
                    ones_sb[0:32, 0:128],
                    start=False,
                    stop=False,
                    skip_group_check=True,
                )

            # ---------------- init ----------------
            # tiny SIN on the ACT queue up front forces the activation-table
            # load to happen during startup instead of delaying the first
            # real SIN
            warm = const_pool.tile([128, 2], f16)
            nc.vector.memset(warm[:, 0:1], 0.25)
            nc.scalar.activation(
                warm[:, 1:2], warm[:, 0:1], mybir.ActivationFunctionType.Sin
            )
            nc.vector.memset(ones_sb[:], 1.0)
            nc.sync.dma_start(wm_sb[:, 0:128], wm_d[:, 0:128])
            nc.sync.dma_start(x_sb[:, 0:1024], x_d[:, 0:1024])
            nc.gpsimd.dma_start(bias_sb[:], bias_d[:])
            nc.gpsimd.dma_start(wm_sb[:, 128:WCOLS], wm_d[:, 128:WCOLS])
            for _ in range(4):
                big_dummy()
            nc.sync.dma_start(x_sb[:, 1024:2048], x_d[:, 1024:2048])

            # ---------------- pixel phase + p0 ----------------
            def do_pix(j):
                # superchunk j: pixels [1024j, 1024j+1024) = rows 16j..16j+16
                # alpha banks alternate (4,5)/(6,7) for 2-deep pipelining
                pix0 = 1024 * j
                b0 = 4 + 2 * (j % 2)
                nc.tensor.matmul(
                    mega[:, b0, :512],
                    wm_sb[0:8, A_ALPHA : A_ALPHA + 128],
                    x_sb[:, pix0 : pix0 + 512],
                    start=True,
                    stop=True,
                    skip_group_check=True,
                )
                nc.tensor.matmul(
                    mega[:, b0 + 1, :512],
                    wm_sb[0:8, A_ALPHA : A_ALPHA + 128],
                    x_sb[:, pix0 + 512 : pix0 + 1024],
                    start=True,
                    stop=True,
                    skip_group_check=True,
                )
                vcs = v_pool.tile([128, 2, 1024], f16, tag="vcs", name=f"vcs{j}")
                nc.scalar.activation(
                    vcs[:, 0, :].opt(), mega[:, b0 : b0 + 2, :512].opt(), SIN,
                    bias=bias_sb[:, 0:1],
                )
                nc.scalar.activation(
                    vcs[:, 1, :].opt(), mega[:, b0 : b0 + 2, :512].opt(), SIN,
                    bias=bias_sb[:, 1:2],
                )
                # triple-angle: phi = ((v^2 - 3/4) * v) * 1, fused DVE op;
                # split per plane so a long CUBE never head-blocks sweep TTs
                # queued behind it on the vector engine
                for pl in range(2):
                    nc.vector._custom_dve(
                        CUBE,
                        out=phi[:, pl, 16 * j : 16 * j + 16, :].opt(),
                        in0=vcs[:, pl, :].opt(),
                        in1=ones_sb[:, 0:1024],
                        s0=0.75,
                    )

            def do_p0(q):
                for ci in range(2):
                    c = 2 * q + ci
                    nr = _chunk_nr(c)
                    nc.tensor.matmul(
                        mega[:, c, : nr * 62],
                        wma(A_P0C),
                        phi[:, 0, 8 * c : 8 * c + nr, 0:62],
                        start=True,
                        stop=False,
                        skip_group_check=True,
                    )
                    nc.tensor.matmul(
                        mega[:, c, : nr * 62],
                        wma(A_P0S),
                        phi[:, 1, 8 * c : 8 * c + nr, 0:62],
                        start=False,
                        stop=True,
                        skip_group_check=True,
                    )



            # ---------------- position sweeps ----------------
            def do_pair(q, p, copy_on_dve=False):
                """Elementwise unit for pair q at position p: one fp16
                downconvert of the pair's 2 chunk banks (pair 3: two copies,
                keeping chunk 7's reads clear of the filler spare region),
                then one 2x-mode TT applying the 2-plane phase multiply.

                copy_on_dve routes the downconvert through the vector engine
                (1x TensorCopy) - used for the first sweep unit so it does
                not queue behind the SIN chain on the saturated ACT queue."""
                dy, dx = p // 3, p % 3
                cs, ce, rows = PAIRS[q]
                r0 = 8 * cs
                ysb = ysb_pool.tile(
                    [128, rows, 62], f16, tag=f"ysb{q}", name=f"y{q}_{p}"
                )
                if copy_on_dve:
                    nc.vector.tensor_copy(
                        ysb[:, :, :].opt(),
                        mega[:, cs : cs + 2, :496].opt(),
                    )
                elif q == 3:
                    nc.scalar.activation(
                        ysb[:, 0:8, :].opt(), mega[:, 6, :496], COPY
                    )
                    nc.scalar.activation(
                        ysb[:, 8:14, :].opt(), mega[:, 7, :372], COPY
                    )
                else:
                    nc.scalar.activation(
                        ysb[:, :, :].opt(),
                        mega[:, cs : cs + 2, :496].opt(),
                        COPY,
                    )
                m = m_pool.tile(
                    [128, 2, rows, 62], f16, tag=f"m{q}", name=f"m{q}_{p}"
                )
                in0 = phi[:, :, r0 + dy : r0 + dy + rows, dx : dx + 62]
                in1 = (
                    ysb[:, :, :].unsqueeze(1).broadcast_to([128, 2, rows, 62])
                )
                nc.vector.tensor_mul(m[:, :, :, :], in0, in1)
                return m

            def do_mms(q, p, m):
                cs, ce, rows = PAIRS[q]
                col = A_M + (p - 1) * 256
                for c in range(cs, ce):
                    nr = _chunk_nr(c)
                    ro = 8 * (c - cs)
                    nc.tensor.matmul(
                        mega[:, c, : nr * 62],
                        wma(col),
                        m[:, 0, ro : ro + nr, :].opt(),
                        start=True,
                        stop=False,
                        skip_group_check=True,
                    )
                    nc.tensor.matmul(
                        mega[:, c, : nr * 62],
                        wma(col + 128),
                        m[:, 1, ro : ro + nr, :].opt(),
                        start=False,
                        stop=True,
                        skip_group_check=True,
                    )

            def do_meas(c):
                nr = _chunk_nr(c)
                C = nr * 62
                c0 = _chunk_c0(c)
                sq = sq_pool.tile([128, 8, 62], f16, tag="sq", name=f"sq{c}")
                if c % 2 == 0:
                    nc.scalar.activation(
                        sq[:, :nr, :].opt(), mega[:, c, :C], SQUARE
                    )
                else:
                    nc.vector._custom_dve(
                        SQOP, out=sq[:, :nr, :].opt(), in0=mega[:, c, :C]
                    )
                nc.tensor.matmul(
                    mega[0:16, c, :C],
                    wma(A_Z, 16),
                    sq[:, :nr, :].opt(),
                    start=True,
                    stop=True,
                    skip_group_check=True,
                )
                if c % 2 == 1:
                    nc.scalar.activation(
                        zbuf[:, c0 : c0 + C], mega[0:16, c, :C], COPY
                    )
                else:
                    nc.vector.tensor_copy(
                        zbuf[:, c0 : c0 + C], mega[0:16, c, :C]
                    )
                nc.sync.dma_start(z_d[:, c0 : c0 + C], zbuf[:, c0 : c0 + C])

            # ------------- emission: lockstep rounds, pair 1 last ----------
            # Pair 1 starts last so banks 2/3 stay free for full-width PE
            # filler matmuls through the pixel phase; pair 0's first sweep
            # unit is pre-emitted to overlap the pixel tail.
            def unit(q, p, dum, copy_on_dve=False):
                m = do_pair(q, p, copy_on_dve)
                do_mms(q, p, m)
                n, mk = dum
                for _ in range(n):
                    mk()
                if p == NPOS - 1:
                    do_meas(2 * q)
                    do_meas(2 * q + 1)

            do_pix(0)
            for _ in range(PIX_DUM):
                big_dummy()
            nc.sync.dma_start(x_sb[:, 2048:3072], x_d[:, 2048:3072])
            do_pix(1)
            for _ in range(PIX_DUM):
                big_dummy()
            do_p0(0)
            nc.sync.dma_start(x_sb[:, 3072:4096], x_d[:, 3072:4096])
            do_pix(2)
            for _ in range(PIX_DUM):
                big_dummy()
            unit(0, 1, (PIX_DUM, big_dummy), copy_on_dve=True)
            do_pix(3)
            for _ in range(PIX_DUM):
                big_dummy2()
            do_p0(2)
            do_p0(3)
            do_p0(1)
            for p in range(1, NPOS):
                for q in (0, 2, 3, 1):
                    if (q, p) == (0, 1):
                        continue
                    unit(q, p, (SWEEP_DUM, small_dummy))

    nc.compile()
    _PROGRAM_CACHE[key] = nc
    return nc


# ---------------- entry point ----------------
_BIAS32 = np.zeros((128, 2), dtype=np.float32)
_BIAS32[:, 0] = np.pi / 6


def kernel(x, weights):
    x = np.asarray(x, dtype=np.float32)
    weights = np.asarray(weights, dtype=np.float32)
    wm = _build_wm(weights)

    nc = _build_program()
    in_maps = []
    for c in range(N_CORES):
        xc = np.ascontiguousarray(
            x[2 * c : 2 * c + 2].reshape(8, NPIX), dtype=np.float16
        )
        in_maps.append({"xin": xc, "wm": wm, "bias32": _BIAS32})
    res = run_bass_kernel_spmd(nc, in_maps, list(range(N_CORES)))

    out = np.zeros((B, 2 * IN_CH, OH, OH), dtype=np.float32)
    for c in range(N_CORES):
        z = np.asarray(res.results[c]["zout"]).astype(np.float32)  # (16, P)
        for k in range(2):
            for img in range(2):
                b = 2 * c + img
                for q in range(IN_CH):
                    out[b, k * IN_CH + q] = z[k * 8 + img * 4 + q].reshape(OH, OH)
    return out


# revision 68
# speedup vs baseline: 1.0137x; 1.0137x over previous
"""Trainium2 Bass kernel for nn_ChannelWiseQuantumKernel (v3).

Method (per-pixel phase precompute + compact z/w state recurrence):

The per-position data RY gates are diagonal phase gates in the (SH)-transformed
basis: the 16 phase angles per patch-position are SHIFTED READS of a per-PIXEL
field alpha_j(img, h, w) = sum_ch +-theta_ch/2 (a 4->16 signed channel mix).
All trig is computed once per pixel: PhiC = -cos(alpha)/4, PhiS = -sin(alpha)/4
stored as fp16 pixel arrays [128, 2, 64, 64] (partitions = kernel(2) x img(2)
x state(32)).

State: compact 32 real values per (kernel, img): y = [yR(16); yI(16)], all
four (k, img) groups packed in 128 partitions. Per position p:
    z = PhiC_view(p) * y ;  w = PhiS_view(p) * y      (one DVE TT, fp16 2x
                                                       mode, broadcast y)
    y' = M1_p @ z + M2_p @ w                          (two accumulated fp16
                                                       128x128 matmuls)

v3 structural changes over the 88us v2 baseline (measured ~80-85us):
 * ALL PSUM lives in one [128, 8, 512] fp32 mega-tile (bank c = chunk c's
   state accumulator, in-place across positions via subtile hazards).
 * 62 output rows split into 8 chunks of 8 rows (chunk 7: 6); chunk PAIRS
   form 4 independent chains, each sharing one ACT downconvert (a single
   [128, 2banks, 496] -> fp16 copy) and one 16-row 2x-mode DVE multiply,
   halving per-instruction init overhead vs per-chunk ops. Chains run in
   lockstep rounds (q order 0,2,3,1) - engine-throughput-bound, with chain
   latency (~3.5us) hidden across rounds.
 * The PE p-state is held at full clock (2.4 GHz, 207ns/496-col matmul
   instead of 413ns at the mid p-state) by a continuous stream of filler
   matmuls: any PE idle gap resets the 3us ramp, so the fillers are
   load-bearing. 496-col fillers go to bank 3 (pair 1 starts last) during
   the pixel phase; 128-col accumulate-only fillers go to the spare cols
   [384:512] of bank 7 (chunk 7 only uses 372) during the sweeps.
 * Pixel phase runs in 4 superchunks (1024 px): 2 alpha matmuls into
   alternating bank pairs (4,5)/(6,7) for 2-deep pipelining, 2 merged
   [128,1024] SIN activations, 1 merged CUBE; a warm-up SIN at t=0 pulls
   the activation-table load out of the critical path. Pair 0's first
   sweep unit is emitted mid-pixel-phase to overlap the pixel tail.
 * GpSimd runs NO compute: it shares SBUF read/write ports with the DVE,
   so its TTs would stall vector work (only DMA kicks go there).
 * Measurement (square -> +-1 matmul -> fp16 copy -> DMA) is emitted inside
   the last round per pair and alternates ACT/DVE for the square/copy.
"""

import sys

sys.path.insert(0, "/opt/trn_rl_repo")

import numpy as np

import concourse.bacc as bacc
import concourse.bass as bass
import concourse.tile as tile
from concourse import mybir
from concourse.bass_utils import run_bass_kernel_spmd

# ---------------- problem constants ----------------
IN_CH = 4
KSZ = 3
NPOS = 9
DIM = 16
B = 16
HW = 64
OH = HW - KSZ + 1  # 62
P = OH * OH  # 3844 patches per image
N_CORES = 8
NPIX = HW * HW  # 4096
NCH = 8  # 8 chunks of 8 output rows (chunk 7 has 6 valid rows)
NPAIR = 4

# knobs
PIX_DUM = 7  # 496-col PE fillers per pixel-superchunk slot
SWEEP_DUM = 5  # 128-col PE fillers after each pair's matmuls
# pair units: (first chunk, end chunk, rows); 4 independent chains
PAIRS = [(0, 2, 16), (2, 4, 16), (4, 6, 16), (6, 8, 14)]

# wm (fp16 weight matrix) column layout
A_ALPHA = 0
A_P0C = 128
A_P0S = 256
A_M = 384  # 16 x 128 cols: (p-1)*256 + {0 (M1), 128 (M2)}
A_Z = A_M + 16 * 128  # 2432
WCOLS = A_Z + 16  # 2448

# ---------------- host-side constant math (weights-only, O(1)) ----------------
_H = np.array([[1, 1], [1, -1]], dtype=np.complex128) / np.sqrt(2)
_S = np.array([[1, 0], [0, 1j]], dtype=np.complex128)
_A1 = _S @ _H


def _kron_n(mats):
    out = np.array([[1.0 + 0j]])
    for m in mats:
        out = np.kron(out, m)
    return out


_AA = _kron_n([_A1] * IN_CH)
_U1 = _A1.conj().T @ (np.array([1.0, 1.0]) / np.sqrt(2))
_YINIT = _kron_n([_U1.reshape(2, 1)] * IN_CH).reshape(DIM)

_SGN = np.array(
    [[2 * ((i >> (3 - ch)) & 1) - 1 for i in range(DIM)] for ch in range(IN_CH)],
    dtype=np.float64,
)
_SIGMA = np.array(
    [[1 - 2 * ((i >> (3 - q)) & 1) for i in range(DIM)] for q in range(IN_CH)],
    dtype=np.float64,
)


def _rx(t):
    c, s = np.cos(t / 2), np.sin(t / 2)
    return np.array([[c, -1j * s], [-1j * s, c]])


def _ry(t):
    c, s = np.cos(t / 2), np.sin(t / 2)
    return np.array([[c, -s], [s, c]])


def _rz(t):
    e = np.exp(-0.5j * t)
    return np.array([[e, 0], [0, np.conj(e)]])


def _embed(U, q):
    mats = [np.eye(2, dtype=complex)] * IN_CH
    mats[q] = U
    return _kron_n(mats)


def _cx(cq, tq):
    M = np.zeros((DIM, DIM), dtype=complex)
    for i in range(DIM):
        bits = [(i >> (3 - q)) & 1 for q in range(4)]
        j = i
        if bits[cq] == 1:
            bits2 = bits.copy()
            bits2[tq] ^= 1
            j = sum(b << (3 - q) for q, b in enumerate(bits2))
        M[j, i] = 1
    return M


def _build_G(w_flat):
    w = np.float64(w_flat).reshape(NPOS, 1, IN_CH, 3)
    Gs = []
    for pos in range(NPOS):
        U = np.eye(DIM, dtype=complex)
        for q in range(IN_CH):
            R = _rz(w[pos, 0, q, 2]) @ _ry(w[pos, 0, q, 1]) @ _rx(w[pos, 0, q, 0])
            U = _embed(R, q) @ U
        for q in range(IN_CH - 1):
            U = _cx(q, q + 1) @ U
        U = _cx(IN_CH - 1, 0) @ U
        Gs.append(_AA.conj().T @ U @ _AA)
    G8f = _AA @ Gs[8]
    return Gs, G8f


def _build_wm(weights):
    """All matmul lhsT constants, fp16 [128, WCOLS].

    Partition groups g = k*2 + img occupy partitions [32g, 32g+32)."""
    wm = np.zeros((128, WCOLS), dtype=np.float64)
    # alpha lhsT: rows (img*4+ch) 0..8, cols (g, j32); emits alpha/3 directly
    for k in range(2):
        for img in range(2):
            g = k * 2 + img
            for ch in range(IN_CH):
                for j in range(32):
                    wm[img * 4 + ch, A_ALPHA + g * 32 + j] = _SGN[ch, j % 16] / 6.0
    for k in range(2):
        Gs, G8f = _build_G(weights[k])
        G0c = Gs[0] @ np.diag(_YINIT)
        Pm, Qm = G0c.real, G0c.imag
        Ac = np.vstack([Pm, Qm])  # 32x16: c16 -> [y1R; y1I]
        As = np.vstack([-Qm, Pm])
        for img in range(2):
            g = k * 2 + img
            r0 = g * 32
            # pos0: lhsT[r0+j, A_P0C + r0+o] = -4*Ac[o, j] (j<16)
            wm[r0 : r0 + 16, A_P0C + r0 : A_P0C + r0 + 32] = -4.0 * Ac.T
            wm[r0 : r0 + 16, A_P0S + r0 : A_P0S + r0 + 32] = -4.0 * As.T
        for p in range(1, 9):
            G = G8f if p == 8 else Gs[p]
            Gr, Gi = G.real, G.imag
            M1 = np.block([[Gr, -Gi], [Gi, Gr]])
            M2 = np.block([[-Gi, -Gr], [Gr, -Gi]])
            col = A_M + (p - 1) * 256
            for img in range(2):
                g = k * 2 + img
                r0 = g * 32
                wm[r0 : r0 + 32, col + r0 : col + r0 + 32] = -4.0 * M1.T
                wm[r0 : r0 + 32, col + 128 + r0 : col + 128 + r0 + 32] = -4.0 * M2.T
        # Z lhsT: out row r = k*8 + img*4 + q
        for img in range(2):
            g = k * 2 + img
            r0 = g * 32
            for q in range(IN_CH):
                for j in range(32):
                    wm[r0 + j, A_Z + k * 8 + img * 4 + q] = _SIGMA[q, j % 16]
    return wm.astype(np.float16)


# ---------------- custom fused DVE op: out = ((v^2 - 3/4) * v) * y ----------
_CUBE_OP = None


def _register_cube_mul():
    """Register the fused triple-angle multiply as a custom DVE op.

    One VectorE instruction computes ((v*v - c0) * v) * y, which applies the
    full range-reduced phase factor to the state in a single pass."""
    global _CUBE_OP
    if _CUBE_OP is not None:
        return _CUBE_OP
    import concourse.dve_ops as dve_ops

    for o in dve_ops.OPS:
        if o.name == "CUBE_MUL_ANT":
            _CUBE_OP = o
            return o
    from concourse.dve_ops import DveOp
    from concourse.dve_spec import C0, Spec, Src0, Src1, lower
    from concourse.dve_uop import DveOpSpec

    body = ((Src0 * Src0 - C0) * Src0) * Src1
    spec = Spec(
        body=body,
        reference=lambda in0, in1, c0, c1, c2: (
            ((in0.astype(np.float32) * in0 - c0) * in0) * in1
        ),
    )
    row = max(dve_ops._SUB_OPCODE_FOR_NAME.values()) + 1
    shas = {}
    for ver in ("v3", "v4"):
        uops = lower(spec, ver=ver)
        shas[ver] = DveOpSpec(
            name="CUBE_MUL_ANT", opcode=row, uops=uops, rd1_en=True
        ).sha(ver)
    op = DveOp("CUBE_MUL_ANT", spec, subdim=False, uops_sha=shas)
    dve_ops.OPS.append(op)
    dve_ops._SUB_OPCODE_FOR_NAME[op.name] = row
    dve_ops.CUSTOM_DVE_SPECS[op.name] = spec
    _CUBE_OP = op
    return op


_SQ_OP = None


def _register_square():
    """Single-source DVE square (PSUM-in, fp16-out) for the measurement."""
    global _SQ_OP
    if _SQ_OP is not None:
        return _SQ_OP
    import concourse.dve_ops as dve_ops

    for o in dve_ops.OPS:
        if o.name == "SQ_ANT":
            _SQ_OP = o
            return o
    from concourse.dve_ops import DveOp
    from concourse.dve_spec import Spec, Src0, lower
    from concourse.dve_uop import DveOpSpec

    spec = Spec(
        body=Src0 * Src0,
        reference=lambda in0, in1, c0, c1, c2: in0.astype(np.float32) ** 2,
    )
    row = max(dve_ops._SUB_OPCODE_FOR_NAME.values()) + 1
    shas = {}
    for ver in ("v3", "v4"):
        uops = lower(spec, ver=ver)
        shas[ver] = DveOpSpec(
            name="SQ_ANT", opcode=row, uops=uops, rd1_en=False
        ).sha(ver)
    op = DveOp("SQ_ANT", spec, subdim=False, uops_sha=shas)
    dve_ops.OPS.append(op)
    dve_ops._SUB_OPCODE_FOR_NAME[op.name] = row
    dve_ops.CUSTOM_DVE_SPECS[op.name] = spec
    _SQ_OP = op
    return op


# ---------------- device program ----------------
_PROGRAM_CACHE = {}

f16 = mybir.dt.float16
f32 = mybir.dt.float32


def _chunk_nr(c):
    return 6 if c == NCH - 1 else 8


def _chunk_c0(c):
    return 496 * c


def _build_program():
    key = "v3"
    if key in _PROGRAM_CACHE:
        return _PROGRAM_CACHE[key]

    nc = bacc.Bacc("TRN2", target_bir_lowering=False, debug=False)
    x_d = nc.dram_tensor("xin", [8, NPIX], f16, kind="ExternalInput").ap()
    wm_d = nc.dram_tensor("wm", [128, WCOLS], f16, kind="ExternalInput").ap()
    bias_d = nc.dram_tensor("bias32", [128, 2], f32, kind="ExternalInput").ap()
    z_d = nc.dram_tensor("zout", [16, P], f16, kind="ExternalOutput").ap()

    CUBE = _register_cube_mul()
    SQOP = _register_square()
    SIN = mybir.ActivationFunctionType.Sin
    COPY = mybir.ActivationFunctionType.Copy
    SQUARE = mybir.ActivationFunctionType.Square

    with tile.TileContext(nc) as tc:
        from contextlib import ExitStack

        with ExitStack() as ctx:
            const_pool = ctx.enter_context(tc.tile_pool(name="const", bufs=1))
            v_pool = ctx.enter_context(tc.tile_pool(name="v", bufs=2))
            ysb_pool = ctx.enter_context(tc.tile_pool(name="ysb", bufs=4))
            m_pool = ctx.enter_context(tc.tile_pool(name="m", bufs=4))
            sq_pool = ctx.enter_context(tc.tile_pool(name="sq", bufs=2))
            ps_pool = ctx.enter_context(
                tc.tile_pool(name="ps", bufs=1, space="PSUM")
            )

            wm_sb = const_pool.tile([128, WCOLS], f16)
            x_sb = const_pool.tile([8, NPIX], f16)
            bias_sb = const_pool.tile([128, 2], f32)
            ones_sb = const_pool.tile([128, 2048], f16)
            # Phi pixel arrays: plane 0 = -cos(alpha)/4, plane 1 = -sin/4
            phi = const_pool.tile([128, 2, HW, HW], f16)
            zbuf = const_pool.tile([16, P], f16)
            # ALL of PSUM: bank c = chunk c state accumulator
            mega = ps_pool.tile([128, NCH, 512], f32)

            def wma(c0, n=128):
                return wm_sb[:, c0 : c0 + n]

            def big_dummy():
                # full-col filler into bank 3 (unowned until pair-1 p0,
                # which is deliberately scheduled last)
                nc.tensor.matmul(
                    mega[:, 3, :496],
                    ones_sb[:, 0:128],
                    ones_sb[:, 0:496],
                    start=True,
                    stop=True,
                    skip_group_check=True,
                )

            def big_dummy2():
                # filler into bank 2 during the late pixel window
                nc.tensor.matmul(
                    mega[:, 2, :496],
                    ones_sb[:, 0:128],
                    ones_sb[:, 0:496],
                    start=True,
                    stop=True,
                    skip_group_check=True,
                )

            def small_dummy():
                # accumulate-only filler into bank 7 spare cols; 32-row
                # stationary keeps the implicit weight load short
                nc.tensor.matmul(
                    mega[:, 7, 384:512],
                    ones_sb[0:32, 0:128],
                    ones_sb[0:32, 0:128],
                    start=False,
                    stop=False,
                    skip_group_check=True,
                )

            # ---------------- init ----------------
            # tiny SIN on the ACT queue up front forces the activation-table
            # load to happen during startup instead of delaying the first
            # real SIN
            warm = const_pool.tile([128, 2], f16)
            nc.vector.memset(warm[:, 0:1], 0.25)
            nc.scalar.activation(
                warm[:, 1:2], warm[:, 0:1], mybir.ActivationFunctionType.Sin
            )
            nc.vector.memset(ones_sb[:], 1.0)
            nc.sync.dma_start(wm_sb[:, 0:128], wm_d[:, 0:128])
            nc.sync.dma_start(x_sb[:, 0:1024], x_d[:, 0:1024])
            nc.gpsimd.dma_start(bias_sb[:], bias_d[:])
            nc.gpsimd.dma_start(wm_sb[:, 128:WCOLS], wm_d[:, 128:WCOLS])
            for _ in range(4):
                big_dummy()
            nc.sync.dma_start(x_sb[:, 1024:2048], x_d[:, 1024:2048])

            # ---------------- pixel phase + p0 ----------------
            def do_pix(j):
                # superchunk j: pixels [1024j, 1024j+1024) = rows 16j..16j+16
                # alpha banks alternate (4,5)/(6,7) for 2-deep pipelining
                pix0 = 1024 * j
                b0 = 4 + 2 * (j % 2)
                nc.tensor.matmul(
                    mega[:, b0, :512],
                    wm_sb[0:8, A_ALPHA : A_ALPHA + 128],
                    x_sb[:, pix0 : pix0 + 512],
                    start=True,
                    stop=True,
                    skip_group_check=True,
                )
                nc.tensor.matmul(
                    mega[:, b0 + 1, :512],
                    wm_sb[0:8, A_ALPHA : A_ALPHA + 128],
                    x_sb[:, pix0 + 512 : pix0 + 1024],
                    start=True,
                    stop=True,
                    skip_group_check=True,
                )
                vcs = v_pool.tile([128, 2, 1024], f16, tag="vcs", name=f"vcs{j}")
                nc.scalar.activation(
                    vcs[:, 0, :].opt(), mega[:, b0 : b0 + 2, :512].opt(), SIN,
                    bias=bias_sb[:, 0:1],
                )
                nc.scalar.activation(
                    vcs[:, 1, :].opt(), mega[:, b0 : b0 + 2, :512].opt(), SIN,
                    bias=bias_sb[:, 1:2],
                )
                # triple-angle: phi = ((v^2 - 3/4) * v) * 1, fused DVE op;
                # split per plane so a long CUBE never head-blocks sweep TTs
                # queued behind it on the vector engine
                for pl in range(2):
                    nc.vector._custom_dve(
                        CUBE,
                        out=phi[:, pl, 16 * j : 16 * j + 16, :].opt(),
                        in0=vcs[:, pl, :].opt(),
                        in1=ones_sb[:, 0:1024],
                        s0=0.75,
                    )

            def do_p0(q):
                for ci in range(2):
                    c = 2 * q + ci
                    nr = _chunk_nr(c)
                    nc.tensor.matmul(
                        mega[:, c, : nr * 62],
                        wma(A_P0C),
                        phi[:, 0, 8 * c : 8 * c + nr, 0:62],
                        start=True,
                        stop=False,
                        skip_group_check=True,
                    )
                    nc.tensor.matmul(
                        mega[:, c, : nr * 62],
                        wma(A_P0S),
                        phi[:, 1, 8 * c : 8 * c + nr, 0:62],
                        start=False,
                        stop=True,
                        skip_group_check=True,
                    )



            # ---------------- position sweeps ----------------
            def do_pair(q, p, copy_on_dve=False):
                """Elementwise unit for pair q at position p: one fp16
                downconvert of the pair's 2 chunk banks (pair 3: two copies,
                keeping chunk 7's reads clear of the filler spare region),
                then one 2x-mode TT applying the 2-plane phase multiply.

                copy_on_dve routes the downconvert through the vector engine
                (1x TensorCopy) - used for the first sweep unit so it does
                not queue behind the SIN chain on the saturated ACT queue."""
                dy, dx = p // 3, p % 3
                cs, ce, rows = PAIRS[q]
                r0 = 8 * cs
                ysb = ysb_pool.tile(
                    [128, rows, 62], f16, tag=f"ysb{q}", name=f"y{q}_{p}"
                )
                if copy_on_dve:
                    nc.vector.tensor_copy(
                        ysb[:, :, :].opt(),
                        mega[:, cs : cs + 2, :496].opt(),
                    )
                elif q == 3:
                    nc.scalar.activation(
                        ysb[:, 0:8, :].opt(), mega[:, 6, :496], COPY
                    )
                    nc.scalar.activation(
                        ysb[:, 8:14, :].opt(), mega[:, 7, :372], COPY
                    )
                else:
                    nc.scalar.activation(
                        ysb[:, :, :].opt(),
                        mega[:, cs : cs + 2, :496].opt(),
                        COPY,
                    )
                m = m_pool.tile(
                    [128, 2, rows, 62], f16, tag=f"m{q}", name=f"m{q}_{p}"
                )
                in0 = phi[:, :, r0 + dy : r0 + dy + rows, dx : dx + 62]
                in1 = (
                    ysb[:, :, :].unsqueeze(1).broadcast_to([128, 2, rows, 62])
                )
                nc.vector.tensor_mul(m[:, :, :, :], in0, in1)
                return m

            def do_mms(q, p, m):
                cs, ce, rows = PAIRS[q]
                col = A_M + (p - 1) * 256
                for c in range(cs, ce):
                    nr = _chunk_nr(c)
                    ro = 8 * (c - cs)
                    nc.tensor.matmul(
                        mega[:, c, : nr * 62],
                        wma(col),
                        m[:, 0, ro : ro + nr, :].opt(),
                        start=True,
                        stop=False,
                        skip_group_check=True,
                    )
                    nc.tensor.matmul(
                        mega[:, c, : nr * 62],
                        wma(col + 128),
                        m[:, 1, ro : ro + nr, :].opt(),
                        start=False,
                        stop=True,
                        skip_group_check=True,
                    )

            def do_meas(c):
                nr = _chunk_nr(c)
                C = nr * 62
                c0 = _chunk_c0(c)
                sq = sq_pool.tile([128, 8, 62], f16, tag="sq", name=f"sq{c}")
                if c % 2 == 0:
                    nc.scalar.activation(
                        sq[:, :nr, :].opt(), mega[:, c, :C], SQUARE
                    )
                else:
                    nc.vector._custom_dve(
                        SQOP, out=sq[:, :nr, :].opt(), in0=mega[:, c, :C]
                    )
                nc.tensor.matmul(
                    mega[0:16, c, :C],
                    wma(A_Z, 16),
                    sq[:, :nr, :].opt(),
                    start=True,
                    stop=True,
                    skip_group_check=True,
                )
                if c % 2 == 1:
                    nc.scalar.activation(
                        zbuf[:, c0 : c0 + C], mega[0:16, c, :C], COPY
                    )
                else:
                    nc.vector.tensor_copy(
                        zbuf[:, c0 : c0 + C], mega[0:16, c, :C]
                    )
                nc.sync.dma_start(z_d[:, c0 : c0 + C], zbuf[:, c0 : c0 + C])

            # ------------- emission: lockstep rounds, pair 1 last ----------
            # Pair 1 starts last so banks 2/3 stay free for full-width PE
            # filler matmuls through the pixel phase; pair 0's first sweep
            # unit is pre-emitted to overlap the pixel tail.
            def unit(q, p, dum, copy_on_dve=False):
                m = do_pair(q, p, copy_on_dve)
                do_mms(q, p, m)
                n, mk = dum
                for _ in range(n):
                    mk()
                if p == NPOS - 1:
                    do_meas(2 * q)
                    do_meas(2 * q + 1)

            do_pix(0)
            for _ in range(PIX_DUM):
                big_dummy()
            nc.sync.dma_start(x_sb[:, 2048:3072], x_d[:, 2048:3072])
            do_pix(1)
            for _ in range(PIX_DUM):
                big_dummy()
            do_p0(0)
            nc.sync.dma_start(x_sb[:, 3072:4096], x_d[:, 3072:4096])
            do_pix(2)
            for _ in range(PIX_DUM):
                big_dummy()
            unit(0, 1, (PIX_DUM, big_dummy))
            do_pix(3)
            for _ in range(PIX_DUM):
                big_dummy2()
            do_p0(2)
            do_p0(3)
            do_p0(1)
            for p in range(1, NPOS):
                for q in (0, 2, 3, 1):
                    if (q, p) == (0, 1):
                        continue
                    unit(q, p, (SWEEP_DUM, small_dummy))

    nc.compile()
    _PROGRAM_CACHE[key] = nc
    return nc


# ---------------- entry point ----------------
_BIAS32 = np.zeros((128, 2), dtype=np.float32)
_BIAS32[:, 0] = np.pi / 6


def kernel(x, weights):
    x = np.asarray(x, dtype=np.float32)
    weights = np.asarray(weights, dtype=np.float32)
    wm = _build_wm(weights)

    nc = _build_program()
    in_maps = []
    for c in range(N_CORES):
        xc = np.ascontiguousarray(
            x[2 * c : 2 * c + 2].reshape(8, NPIX), dtype=np.float16
        )
        in_maps.append({"xin": xc, "wm": wm, "bias32": _BIAS32})
    res = run_bass_kernel_spmd(nc, in_maps, list(range(N_CORES)))

    out = np.zeros((B, 2 * IN_CH, OH, OH), dtype=np.float32)
    for c in range(N_CORES):
        z = np.asarray(res.results[c]["zout"]).astype(np.float32)  # (16, P)
        for k in range(2):
            for img in range(2):
                b = 2 * c + img
                for q in range(IN_CH):
                    out[b, k * IN_CH + q] = z[k * 8 + img * 4 + q].reshape(OH, OH)
    return out
